# revision 34
# baseline (speedup 1.0000x reference)
"""Dense GAT (8 heads + classifier) on 8 Trainium2 NeuronCores.

Row-parallel sharding: core m owns output rows [m*750, (m+1)*750).
Each core recomputes the full per-head hidden h = X @ W0[h], computes
masked-softmax attention for its 750 rows against all 6000 columns,
then the classifier layer after a small fp16 AllGather of the
classifier hidden.

Math: exp(leaky_relu(f1_i + f2_j)) = p_i * max(w_i * v_j, q_j)
  with w=exp(0.8 f1), v=exp(f2), q=exp(0.2 f2); p_i constant per
  output row, cancels in the softmax.
So the per-tile elementwise work is:
  tmp = (wb * v_col) max q_col   (tensor_scalar, 2 per-partition
                                  scalars -> 4x DVE mode)
  mpp = tmp * adjT               (tensor_tensor fp16 -> 2x DVE mode)
with ONE raw adjacency mask tile shared by all 8 heads AND the
classifier (the per-head v_j scaling lives in the scalars, not the
mask), so mask DMA is 9 MB once instead of 81 MB.
"""
import sys
sys.path.insert(0, "/opt/trn_rl_repo")
import numpy as np
import ml_dtypes

import concourse.bass as bass
import concourse.bacc as bacc
import concourse.mybir as mybir
from concourse import tile
from concourse.bass_utils import run_bass_kernel_spmd

F32 = mybir.dt.float32
F32R = mybir.dt.float32r
BF16 = mybir.dt.bfloat16
F16 = mybir.dt.float16
AF = mybir.ActivationFunctionType
ALU = mybir.AluOpType

N, D, F, H, C = 6000, 512, 256, 8, 32
NCORES = 8
NSH = N // NCORES            # 750 rows per core
P = 128
KT = (N + P - 1) // P        # 47 j-tiles; last has 112 rows
GW = F                       # 256: h values only (denom via ones matmul)
CW = C + 2                   # 34: classifier vals + f2c col + pad
DT = D // P                  # 4 contraction tiles for h-matmul
FT = (H * F) // P            # 16 xT partition tiles
CORE_IDS = list(range(NCORES))
# even-width output row chunks: 5x126 + 120 = 750
CHUNKS = [(i * 126, min((i + 1) * 126, NSH)) for i in range(6)]
HALVES = [(0, 376), (376, NSH)]  # even halves for row-vector matmuls
NHALF = 23 * P               # featT half split, j-tile aligned (2944)


def _jn(jt):
    return min(P, N - jt * P)


def build():
    nc = bacc.Bacc("TRN2", target_bir_lowering=False, debug=False,
                   num_devices=NCORES)

    featT_d = nc.dram_tensor("featT", [D, N], F32R, kind="ExternalInput")
    w0_d = nc.dram_tensor("w0", [P, H * DT * F], F32R, kind="ExternalInput")
    wb_d = nc.dram_tensor("wb", [H * P, NSH], F16, kind="ExternalInput")
    vq_d = nc.dram_tensor("vq", [P, KT * 2 * H], F32, kind="ExternalInput")
    adjT_d = nc.dram_tensor("adjT", [P, KT * NSH], F16, kind="ExternalInput")
    wcx_d = nc.dram_tensor("wcx", [P, FT * CW], F16, kind="ExternalInput")
    w1c_d = nc.dram_tensor("w1c", [P, FT * 2], F16, kind="ExternalInput")
    ident_d = nc.dram_tensor("ident", [P, P], F16, kind="ExternalInput")
    out_d = nc.dram_tensor("O", [NSH, C], F32, kind="ExternalOutput")

    import os
    with tile.TileContext(nc, trace_sim=bool(os.environ.get('K_TRACE_SIM'))) as tc:
        with (
            tc.tile_pool(name="const", bufs=1) as cpool,
            tc.tile_pool(name="xt", bufs=1) as xtpool,
            tc.tile_pool(name="w0", bufs=2) as w0pool,
            tc.tile_pool(name="wbp", bufs=2) as wbpool,
            tc.tile_pool(name="g", bufs=3) as gpool,
            tc.tile_pool(name="xsm", bufs=3) as xpool,
            tc.tile_pool(name="xe", bufs=6) as xepool,
            tc.tile_pool(name="cls", bufs=3) as clpool,
            tc.tile_pool(name="cl2", bufs=5) as clpool2,
            tc.tile_pool(name="adjs", bufs=3) as adjspool,
            tc.tile_pool(name="tmp", bufs=2) as tmppool,
            tc.tile_pool(name="mpp", bufs=2) as mpool,
            tc.tile_pool(name="hps", bufs=3, space="PSUM") as hps,
            tc.tile_pool(name="att", bufs=3, space="PSUM") as attps,
            tc.tile_pool(name="dn", bufs=2, space="PSUM") as dnps,
            tc.tile_pool(name="dram", bufs=1, space="DRAM") as dram,
        ):
            # ---- persistent tiles ----
            # featT in bf16, split into halves so early j-tiles unblock fast
            FPC = [(0, 1536), (1536, NHALF), (NHALF, N)]
            featT = [[cpool.tile([P, b - a], F32R,
                                 tag=f"featT{i}_{hf}", name=f"featT{i}_{hf}")
                      for hf, (a, b) in enumerate(FPC)]
                     for i in range(DT)]
            for hf in range(2):
                a, b = FPC[hf]
                for i in range(DT):
                    eng = nc.gpsimd if i < 2 else nc.scalar
                    eng.dma_start(featT[i][hf][:],
                                  featT_d[i * P:(i + 1) * P, a:b])

            # head-0 weights first so the PE can start ASAP
            w0_h0 = w0pool.tile([P, DT * F], F32R, tag="w0", name="w0_0")
            nc.sync.dma_start(w0_h0[:], w0_d[:, 0:DT * F])
            wb_h0 = wbpool.tile([P, NSH], F16, tag="wb")
            nc.sync.dma_start(wb_h0[:], wb_d[0:P, :])
            ident = cpool.tile([P, P], F16, tag="ident")
            nc.gpsimd.dma_start(ident[:], ident_d[:])
            wcxall = cpool.tile([P, FT * CW], F16, tag="wcx")
            nc.gpsimd.dma_start(wcxall[:], wcx_d[:])
            w1call = cpool.tile([P, FT * 2], F16, tag="w1c")
            nc.gpsimd.dma_start(w1call[:], w1c_d[:])
            for i in (0, 1):
                nc.gpsimd.dma_start(featT[i][2][:],
                                    featT_d[i * P:(i + 1) * P, NHALF:N])
            # const [1|0] columns in f16
            oz = cpool.tile([P, 2], F16, tag="oz")
            nc.vector.memset(oz[:, 0:1], 1.0)
            nc.vector.memset(oz[:, 1:2], 0.0)
            ones_row = cpool.tile([1, P], F16, tag="ones")
            nc.vector.memset(ones_row[:], 1.0)
            # per-j scalars: v=exp(f2), q=exp(0.2 f2) per head (tile layout)
            vq = cpool.tile([P, KT * 2 * H], F32, tag="vq")
            nc.sync.dma_start(vq[:], vq_d[:])
            # raw adjacency mask, resident, shared by all heads+classifier;
            # staged in batches interleaved with featT second halves so
            # availability tracks the head-0 consumption rate
            adjT = cpool.tile([P, 30 * NSH], F16, tag="adjT")
            ADJ_STAGES = [(0, 3), (3, 8), (8, 16), (16, 24), (24, 30)]
            for si, (sa, sb) in enumerate(ADJ_STAGES):
                nc.sync.dma_start(adjT[:, sa * NSH:sb * NSH],
                                  adjT_d[:, sa * NSH:sb * NSH])
                if si == 3:
                    for i in (2, 3):
                        nc.sync.dma_start(featT[i][2][:],
                                          featT_d[i * P:(i + 1) * P, NHALF:N])

            xT = [xtpool.tile([P, NSH], F16, tag=f"xT{i}", name=f"xT{i}")
                  for i in range(FT)]

            # ---- 8 attention heads ----
            for h in range(H):
                if h == 0:
                    w0, wb = w0_h0, wb_h0
                else:
                    w0 = w0pool.tile([P, DT * F], F32R, tag="w0",
                                     name=f"w0_{h}")
                    nc.sync.dma_start(w0[:],
                                      w0_d[:, h * DT * F:(h + 1) * DT * F])
                    wb = wbpool.tile([P, NSH], F16, tag="wb")
                    nc.sync.dma_start(wb[:], wb_d[h * P:(h + 1) * P, :])

                att2 = [attps.tile([P, 512], F32, tag="att",
                                   name=f"att2_{h}_{c}") for c in range(3)]
                attd = dnps.tile([P, 12], F32, tag="attd", name=f"attd_{h}")
                nc.vector.memset(attd[:], 1.0)

                # drain the previous head's PSUM fast: reciprocal (DVE) +
                # scaled copy to fp16 (Pool) per chunk, freeing att2 bufs
                # pre-issue ts/tt for jt 0,1: they depend only on wb/vq,
                # so the DVE does them during the previous head's tail
                adjs = {}
                premp = {}
                for jt in (0, 1):
                    jn = _jn(jt)
                    tmp = tmppool.tile([P, NSH], F16, tag="tmp")
                    nc.vector.tensor_scalar(
                        tmp[:jn, :], wb[:jn, :],
                        vq[:jn, jt * 2 * H + 2 * h:jt * 2 * H + 2 * h + 1],
                        vq[:jn, jt * 2 * H + 2 * h + 1:jt * 2 * H + 2 * h + 2],
                        op0=ALU.mult, op1=ALU.max)
                    mpp = mpool.tile([P, NSH], F16, tag="mpp")
                    nc.vector.tensor_tensor(
                        mpp[:jn, :], tmp[:jn, :],
                        adjT[:jn, jt * NSH:(jt + 1) * NSH], op=ALU.mult)
                    premp[jt] = mpp

                def drain_xh(pv, c):
                    c0, c1 = CHUNKS[c]
                    cw = c1 - c0
                    ps = pv["att2"][c // 2][:, (c % 2) * F:(c % 2) * F + F]
                    xh = xpool.tile([P, F], F16, tag="xh")
                    if c % 2 == 0:
                        nc.vector.tensor_scalar_mul(
                            xh[:cw], ps[:cw, 0:F],
                            pv["sinv"][:cw, 2 * c:2 * c + 1])
                    else:
                        nc.scalar.activation(
                            xh[:cw], ps[:cw, 0:F], AF.Copy,
                            scale=pv["sinv"][:cw, 2 * c:2 * c + 1])
                    pv["xh"][c] = xh

                if h > 0:
                    pv = pipe  # (att2_prev, attd_prev, h-1)
                    pv["xh"] = [None] * 6
                    # one reciprocal over the whole tile: depends on the
                    # group-closing stop matmul, so all reads come after it
                    sinv = xpool.tile([P, 12], F32, tag="sinv")
                    nc.vector.reciprocal(sinv[:], pv["attd"][:, 0:12])
                    pv["sinv"] = sinv
                    drain_xh(pv, 0)
                    drain_xh(pv, 1)
                    pv["xe"] = []

                for jt in range(KT):
                    jn = _jn(jt)
                    j0 = jt * P
                    hf = 0 if j0 + jn <= 1536 else (1 if j0 + jn <= NHALF
                                                     else 2)
                    jo = j0 - (0, 1536, NHALF)[hf]
                    # prefetch streamed mask tiles (jt >= 38) two ahead
                    if 30 <= jt + 2 < KT:
                        t = adjspool.tile([P, NSH], F16, tag="adjs")
                        nc.sync.dma_start(
                            t[:], adjT_d[:, (jt + 2) * NSH:(jt + 3) * NSH])
                        adjs[jt + 2] = t
                    hp = hps.tile([P, F], F32, tag="h")
                    for i in range(DT):
                        nc.tensor.matmul(hp[:jn, :],
                                         lhsT=featT[i][hf][:, jo:jo + jn],
                                         rhs=w0[:, i * F:(i + 1) * F],
                                         start=(i == 0),
                                         stop=(i == DT - 1))
                    g = gpool.tile([P, F], F16, tag="g")
                    nc.scalar.activation(g[:jn, 0:F], hp[:jn, :], AF.Copy)

                    if jt in premp:
                        mpp = premp[jt]
                        if h > 0:
                            drain_xh(pipe, 2 + 2 * jt)
                            drain_xh(pipe, 3 + 2 * jt)
                    else:
                        # tmp = (wb * v_j) max q_j   -- 4x DVE mode
                        tmp = tmppool.tile([P, NSH], F16, tag="tmp")
                        vcol = vq[:jn,
                                  jt * 2 * H + 2 * h:jt * 2 * H + 2 * h + 1]
                        qcol = vq[:jn,
                                  jt * 2 * H + 2 * h + 1:jt * 2 * H + 2 * h + 2]
                        nc.vector.tensor_scalar(tmp[:jn, :], wb[:jn, :],
                                                vcol, qcol,
                                                op0=ALU.mult, op1=ALU.max)
                        # mpp = tmp * adjT   -- 2x DVE mode
                        msk = (adjT[:jn, jt * NSH:(jt + 1) * NSH] if jt < 30
                               else adjs[jt][:jn, :])
                        mpp = mpool.tile([P, NSH], F16, tag="mpp")
                        nc.vector.tensor_tensor(
                            mpp[:jn, :], tmp[:jn, :], msk, op=ALU.mult)
                    # start zeroes the whole 2KB psum bank; the stop must
                    # cover the same partition span as the start, so the
                    # group-closing matmul is always a 126-row chunk: at the
                    # last jt, chunk 5 (120 rows) runs before chunk 4.
                    last = (jt == KT - 1)
                    order = (0, 1, 2, 3, 5, 4) if last else range(6)
                    for c in order:
                        c0, c1 = CHUNKS[c]
                        nc.tensor.matmul(
                            att2[c // 2][:c1 - c0,
                                         (c % 2) * F:(c % 2) * F + F],
                            lhsT=mpp[:jn, c0:c1], rhs=g[:jn, :],
                            start=(jt == 0 and c % 2 == 0),
                            stop=(last and c in (1, 3, 4)))
                        nc.tensor.matmul(
                            attd[:c1 - c0, 2 * c:2 * c + 2],
                            lhsT=mpp[:jn, c0:c1], rhs=oz[:jn, :],
                            start=(jt == 0 and c == 0),
                            stop=(last and c == 4))
                    # previous head's elu, spread across early iterations:
                    # elu(x) = (max(x,0) - 1) + exp(min(x,0))
                    if h > 0 and 4 <= jt < 16 and jt % 2 == 0:
                        c = (jt - 4) // 2
                        c0, c1 = CHUNKS[c]
                        cw = c1 - c0
                        xh = pipe["xh"][c]
                        a = xpool.tile([P, F], F16, tag="xa")
                        nc.vector.tensor_scalar(a[:cw], xh[:cw], 0.0, -1.0,
                                                op0=ALU.max, op1=ALU.add)
                        b = xpool.tile([P, F], F16, tag="xb")
                        nc.gpsimd.tensor_scalar(b[:cw], xh[:cw], 0.0, None,
                                                op0=ALU.min)
                        ec = xpool.tile([P, F], F16, tag="xc")
                        nc.scalar.activation(ec[:cw], b[:cw], AF.Exp)
                        xe = xepool.tile([P, F], F16, tag="xe")
                        nc.gpsimd.tensor_tensor(xe[:cw], a[:cw], ec[:cw],
                                                op=ALU.add)
                        pipe["xe"].append(xe)
                    # previous head's transposes, also spread out
                    if h > 0 and 16 <= jt < 28 and jt % 2 == 0:
                        c = (jt - 16) // 2
                        c0, c1 = CHUNKS[c]
                        cw = c1 - c0
                        xe = pipe["xe"][c]
                        hprev = pipe["h"]
                        for half in range(2):
                            tp = hps.tile([P, F], F16, tag="h")
                            nc.tensor.transpose(
                                tp[:P, 0:cw],
                                xe[:cw, half * P:(half + 1) * P],
                                ident[:cw, :cw])
                            if half == 0:
                                nc.vector.tensor_copy(
                                    xT[hprev * 2 + half][:, c0:c1],
                                    tp[:P, 0:cw])
                            else:
                                nc.scalar.activation(
                                    xT[hprev * 2 + half][:, c0:c1],
                                    tp[:P, 0:cw], AF.Copy)
                pipe = {"att2": att2, "attd": attd, "h": h}

            # epilogue fused with classifier h_c per chunk, so the gather
            # input streams out while later chunks still normalize
            gin = dram.tile([NSH, CW], F16)
            gout = dram.tile([N, CW], F16, addr_space="Shared")
            sinv_ep = xpool.tile([P, 12], F32, tag="sinv")
            nc.vector.reciprocal(sinv_ep[:], pipe["attd"][:, 0:12])
            for c, (c0, c1) in enumerate(CHUNKS):
                cw = c1 - c0
                ps = pipe["att2"][c // 2][:, (c % 2) * F:(c % 2) * F + F]
                xh = xpool.tile([P, F], F16, tag="xh")
                if c % 2 == 0:
                    nc.vector.tensor_scalar_mul(xh[:cw], ps[:cw, 0:F],
                                                sinv_ep[:cw, 2 * c:2 * c + 1])
                else:
                    nc.scalar.activation(xh[:cw], ps[:cw, 0:F], AF.Copy,
                                         scale=sinv_ep[:cw, 2 * c:2 * c + 1])
                a = xpool.tile([P, F], F16, tag="xa")
                nc.vector.tensor_scalar(a[:cw], xh[:cw], 0.0, -1.0,
                                        op0=ALU.max, op1=ALU.add)
                b = xpool.tile([P, F], F16, tag="xb")
                nc.gpsimd.tensor_scalar(b[:cw], xh[:cw], 0.0, None,
                                        op0=ALU.min)
                ec = xpool.tile([P, F], F16, tag="xc")
                nc.scalar.activation(ec[:cw], b[:cw], AF.Exp)
                xe = xepool.tile([P, F], F16, tag="xe")
                nc.gpsimd.tensor_tensor(xe[:cw], a[:cw], ec[:cw],
                                        op=ALU.add)
                for half in range(2):
                    tp = hps.tile([P, F], F16, tag="h")
                    nc.tensor.transpose(tp[:P, 0:cw],
                                        xe[:cw, half * P:(half + 1) * P],
                                        ident[:cw, :cw])
                    if half == 0:
                        nc.vector.tensor_copy(
                            xT[(H - 1) * 2 + half][:, c0:c1], tp[:P, 0:cw])
                    else:
                        nc.scalar.activation(
                            xT[(H - 1) * 2 + half][:, c0:c1],
                            tp[:P, 0:cw], AF.Copy)
                hc = hps.tile([P, 384], F32, tag="h", name=f"hc_{c}")
                for i in range(FT):
                    nc.tensor.matmul(hc[:cw, 0:CW], lhsT=xT[i][:, c0:c1],
                                     rhs=wcxall[:, i * CW:(i + 1) * CW],
                                     start=(i == 0),
                                     stop=(i == FT - 1))
                hcs = clpool.tile([P, CW], F16, tag="hcs", name=f"hcs_{c}")
                nc.scalar.activation(hcs[:cw], hc[:cw, 0:CW], AF.Copy)
                nc.sync.dma_start(gin[c0:c1, :], hcs[:cw])
            nc.gpsimd.collective_compute("AllGather", ALU.bypass,
                                         replica_groups=[CORE_IDS],
                                         ins=[gin.opt()], outs=[gout.opt()])

            # f1c row [1, 750] via w1c matmuls on xT; wce = exp(0.8 f1c)
            wce = cpool.tile([1, NSH], F16, tag="wce")
            for h0, h1 in HALVES:
                fr = hps.tile([P, 384], F32, tag="h", name=f"fr_{h0}")
                for i in range(FT):
                    nc.tensor.matmul(fr[0:2, 0:h1 - h0],
                                     lhsT=w1call[:, i * 2:(i + 1) * 2],
                                     rhs=xT[i][:, h0:h1],
                                     start=(i == 0), stop=(i == FT - 1))
                nc.scalar.activation(wce[0:1, h0:h1], fr[0:1, 0:h1 - h0],
                                     AF.Exp, scale=0.8)
            wbc = cpool.tile([P, NSH], F16, tag="wbc")
            for h0, h1 in HALVES:
                wp = hps.tile([P, 384], F32, tag="h", name=f"wp_{h0}")
                nc.tensor.matmul(wp[:, 0:h1 - h0], lhsT=ones_row[:],
                                 rhs=wce[0:1, h0:h1], start=True, stop=True)
                nc.scalar.activation(wbc[:, h0:h1], wp[:, 0:h1 - h0], AF.Copy)

            attc = attps.tile([P, 512], F32, tag="att", name="attc")
            attcd = dnps.tile([P, 12], F32, tag="attd", name="attcd")
            nc.vector.memset(attcd[:], 1.0)
            adjsc = {}
            dmaq = [nc.scalar, nc.gpsimd]
            for jt in range(KT):
                jn = _jn(jt)
                j0 = jt * P
                if 30 <= jt + 2 < KT:
                    t = adjspool.tile([P, NSH], F16, tag="adjs")
                    nc.sync.dma_start(
                        t[:], adjT_d[:, (jt + 2) * NSH:(jt + 3) * NSH])
                    adjsc[jt + 2] = t
                hcall = clpool2.tile([P, CW], F16, tag="hcall")
                if jt < 28:
                    nc.sync.dma_start(hcall[:jn, :], gout[j0:j0 + jn, :])
                else:
                    dmaq[jt % 2].dma_start(hcall[:jn, :], gout[j0:j0 + jn, :])
                vc = clpool2.tile([P, 2], F32, tag="vc")
                nc.scalar.activation(vc[:jn, 0:1], hcall[:jn, C:C + 1], AF.Exp)
                qc = clpool2.tile([P, 2], F32, tag="qc")
                nc.scalar.activation(qc[:jn, 0:1], hcall[:jn, C:C + 1], AF.Exp,
                                     scale=0.2)
                tmpc = tmppool.tile([P, NSH], F16, tag="tmp")
                mc = mpool.tile([P, NSH], F16, tag="mpp")
                mskc = (adjT[:jn, jt * NSH:(jt + 1) * NSH] if jt < 30
                        else adjsc[jt][:jn, :])
                nc.vector.tensor_scalar(tmpc[:jn, :], wbc[:jn, :],
                                        vc[:jn, 0:1], qc[:jn, 0:1],
                                        op0=ALU.mult, op1=ALU.max)
                if jt % 2 == 1:
                    nc.gpsimd.tensor_tensor(
                        mc[:jn, :], tmpc[:jn, :], mskc, op=ALU.mult)
                else:
                    nc.vector.tensor_tensor(
                        mc[:jn, :], tmpc[:jn, :], mskc, op=ALU.mult)
                lastc = (jt == KT - 1)
                orderc = (0, 1, 2, 3, 5, 4) if lastc else range(6)
                for c in orderc:
                    c0, c1 = CHUNKS[c]
                    nc.tensor.matmul(attc[:c1 - c0, C * c:C * c + C],
                                     lhsT=mc[:jn, c0:c1],
                                     rhs=hcall[:jn, 0:C],
                                     start=(jt == 0 and c == 0),
                                     stop=(lastc and c == 4))
                    nc.tensor.matmul(attcd[:c1 - c0, 2 * c:2 * c + 2],
                                     lhsT=mc[:jn, c0:c1],
                                     rhs=oz[:jn, :],
                                     start=(jt == 0 and c == 0),
                                     stop=(lastc and c == 4))
            sinv_c = xpool.tile([P, 12], F32, tag="sinv")
            nc.vector.reciprocal(sinv_c[:], attcd[:, 0:12])
            for c, (c0, c1) in enumerate(CHUNKS):
                cw = c1 - c0
                osb = clpool.tile([P, C], F32, tag="osb")
                if c % 2 == 0:
                    nc.vector.tensor_scalar_mul(
                        osb[:cw], attc[:cw, C * c:C * c + C],
                        sinv_c[:cw, 2 * c:2 * c + 1])
                else:
                    nc.scalar.activation(
                        osb[:cw], attc[:cw, C * c:C * c + C], AF.Copy,
                        scale=sinv_c[:cw, 2 * c:2 * c + 1])
                nc.sync.dma_start(out_d[c0:c1, :], osb[:cw])

    nc.compile()
    return nc


_NC_CACHE = None
_LAST_IN_MAPS = None


def kernel(features, adj, W0, a10, a20, Wc, a1c, a2c):
    global _NC_CACHE, _LAST_IN_MAPS
    features = np.asarray(features, dtype=np.float32)
    adj = np.asarray(adj)
    W0 = np.asarray(W0, dtype=np.float32)
    a10 = np.asarray(a10, dtype=np.float32)
    a20 = np.asarray(a20, dtype=np.float32)
    Wc = np.asarray(Wc, dtype=np.float32)
    a1c = np.asarray(a1c, dtype=np.float32)
    a2c = np.asarray(a2c, dtype=np.float32)

    # ---- host-side precompute (all small linear algebra) ----
    f64 = np.float64
    feat64 = features.astype(f64)
    f1 = np.stack([feat64 @ (W0[h].astype(f64) @ a10[h].astype(f64))
                   for h in range(H)])          # [H, N]
    f2 = np.stack([feat64 @ (W0[h].astype(f64) @ a20[h].astype(f64))
                   for h in range(H)])          # [H, N]
    w_all = np.exp(0.8 * f1)                     # [H, N] destination-row term
    v_all = np.exp(f2)                           # [H, N]
    q_all = np.exp(0.2 * f2)                     # [H, N]

    featT = np.ascontiguousarray(features.T)
    # w0 in tile layout: [128, H*DT*F], head h block = concat_i W0[h][i*128:..]
    w0r = np.empty((P, H * DT * F), dtype=np.float32)
    for h in range(H):
        for i in range(DT):
            w0r[:, (h * DT + i) * F:(h * DT + i + 1) * F] = \
                W0[h][i * P:(i + 1) * P, :]
    wcxf = np.zeros((H * F, CW), dtype=np.float16)
    wcxf[:, 0:C] = Wc.astype(np.float16)
    wcxf[:, C] = (Wc.astype(f64) @ a2c.astype(f64)).astype(np.float16)
    wcx = np.empty((P, FT * CW), dtype=np.float16)
    for i in range(FT):
        wcx[:, i * CW:(i + 1) * CW] = wcxf[i * P:(i + 1) * P, :]
    w1cf = np.zeros((H * F, 2), dtype=np.float16)
    w1cf[:, 0] = (Wc.astype(f64) @ a1c.astype(f64)).astype(np.float16)
    w1c = np.empty((P, FT * 2), dtype=np.float16)
    for i in range(FT):
        w1c[:, i * 2:(i + 1) * 2] = w1cf[i * P:(i + 1) * P, :]
    vqf = np.empty((N, 2 * H), dtype=np.float32)
    vqf[:, 0::2] = v_all.T
    vqf[:, 1::2] = q_all.T
    vq = np.zeros((P, KT * 2 * H), dtype=np.float32)
    for jt in range(KT):
        jn = min(P, N - jt * P)
        vq[:jn, jt * 2 * H:(jt + 1) * 2 * H] = vqf[jt * P:jt * P + jn, :]
    ident = np.eye(P, dtype=np.float16)

    adj_bool = adj > 0

    in_maps = []
    for cid in range(NCORES):
        r0, r1 = cid * NSH, (cid + 1) * NSH
        adjTf = adj_bool[r0:r1].T.astype(np.float16)   # [N, NSH]
        adjT = np.zeros((P, KT * NSH), dtype=np.float16)
        for jt in range(KT):
            jn = min(P, N - jt * P)
            adjT[:jn, jt * NSH:(jt + 1) * NSH] = adjTf[jt * P:jt * P + jn, :]
        wb = np.ascontiguousarray(
            np.broadcast_to(w_all[:, None, r0:r1].astype(np.float16),
                            (H, P, NSH)).reshape(H * P, NSH))
        in_maps.append({
            "featT": featT, "w0": w0r, "wb": wb, "vq": vq, "adjT": adjT,
            "wcx": wcx, "w1c": w1c, "ident": ident,
        })

    _LAST_IN_MAPS = in_maps
    if _NC_CACHE is None:
        _NC_CACHE = build()
    res = run_bass_kernel_spmd(_NC_CACHE, in_maps, CORE_IDS)
    out = np.concatenate([res.results[c]["O"] for c in range(NCORES)], axis=0)
    return out.astype(np.float32)
